# revision 1
# baseline (speedup 1.0000x reference)
"""CrossAttentionGate kernel for Trainium2, 8 NeuronCores.

Problem: B=4 batches of single-head spatial cross-attention:
    q = Wq@gate + bq          [B,64,N]   (N = 64*64 = 4096)
    k = Wk@skip + bk          [B,64,N]
    v = Wv@skip + bv          [B,256,N]
    attn = softmax_j(q^T k)   [B,N,N]
    out = gamma * (v @ attn^T) + skip

Sharding: 8 cores = 4 batches x 2 query-halves. Each core computes its
batch's k/v in full (duplicated across the 2 cores of a batch - cheap)
and attends for its 2048 query positions.

Math simplifications used (exact, up to float rounding):
  - bk drops out: it shifts every logit in a row i by the same constant,
    softmax is invariant.
  - No row-max subtraction: logits are O(+-50) for this input
    distribution, exp() stays well inside fp32 range and the ACT engine's
    spline exp is accurate over that range (measured ~1e-5).
  - bv moves past the softmax: rows of attn sum to 1, so v's bias adds
    gamma*bv[c] to every output pixel - folded into the residual on host.

Layout trick: logits are computed TRANSPOSED (ST[j,i] = sum_d k[d,j]q[d,i])
so that P = exp(ST) feeds the output matmul as the moving operand with
j (the softmax axis) on partitions - no on-chip transpose anywhere.
Softmax denominators come from a ones-column matmul on the PE; the
1/sum (and gamma) scaling is applied to the 256x2048 output instead of
the 4096x2048 P matrix.

All matmuls run in float32r (reduced-precision fp32 PE mode, 4x faster
than fp32, ~1.6e-4 matmul rel err vs ~2.2e-3 for bf16).
"""

import numpy as np

import concourse.bass as bass
import concourse.tile as tile
from concourse import bacc, mybir
from concourse.bass_utils import run_bass_kernel_spmd

F32 = mybir.dt.float32
F32R = mybir.dt.float32r
AF = mybir.ActivationFunctionType
BF16 = mybir.dt.bfloat16
ALU = mybir.AluOpType

B, CG, CS, INTER, H, W = 4, 512, 256, 64, 64, 64
N = H * W            # 4096 spatial positions
NCORES = 8
NI = N // 2          # 2048 query positions per core
NJ = N               # full key/value length per core

EXP_BATCH = 1        # ACT PSUM reads must stay within one PSUM bank


def _build_program(exp_batch=EXP_BATCH, do_sums=True, do_attn=True, repeat=1,
                   hw_loop=0, hw_loop_inner=0, hw_loop_proj=0, no_exp=False, st_bufs=4,
                   p_bufs=6, sums_on_dve=False, pair_st=True, sw_pipe=0, out_bf16=False, out_db=False):
    nc = bacc.Bacc(
        "TRN2", target_bir_lowering=False, debug=False, num_devices=NCORES
    )
    d_gate = nc.dram_tensor("gate", [CG, NI], F32, kind="ExternalInput").ap()
    d_skip = nc.dram_tensor("skip", [CS, NJ], F32, kind="ExternalInput").ap()
    d_skipr = nc.dram_tensor("skipr", [CS, NI], F32, kind="ExternalInput").ap()
    d_wqt = nc.dram_tensor("wqt", [CG, INTER], F32, kind="ExternalInput").ap()
    d_wkt = nc.dram_tensor("wkt", [CS, INTER], F32, kind="ExternalInput").ap()
    d_wvt = nc.dram_tensor("wvt", [CS, CS], F32, kind="ExternalInput").ap()
    d_bq = nc.dram_tensor("bq", [INTER, 1], F32, kind="ExternalInput").ap()
    d_gam = nc.dram_tensor("gam", [1, 1], F32, kind="ExternalInput").ap()
    d_ones_c = nc.dram_tensor("ones_c", [128, 1], F32, kind="ExternalInput").ap()
    d_ones_r = nc.dram_tensor("ones_r", [1, 128], F32, kind="ExternalInput").ap()
    d_out = nc.dram_tensor("out", [CS, NI], F32, kind="ExternalOutput").ap()

    KG = CG // 128   # 4 gate channel tiles
    KS = CS // 128   # 2 skip channel tiles
    JT = NJ // 128   # 32 key tiles
    NT = NI // 512   # 4 query column tiles

    with tile.TileContext(nc) as tc:
        with (
            tc.tile_pool(name="res", bufs=1) as res,      # long-lived tensors
            tc.tile_pool(name="stream", bufs=4) as stream,  # P tiles etc.
            tc.tile_pool(name="epi", bufs=2) as epi,
        ):
            # ---- load everything (inputs cast to f32r via DMA bitcast) ----
            # weights first: they unblock the projection matmuls
            wqt_t = []
            for kk in range(KG):
                t = res.tile([128, INTER], F32R, tag=f"wqt{kk}", name=f"wqt{kk}")
                nc.sync.dma_start(
                    t[:], d_wqt[kk * 128:(kk + 1) * 128, :].bitcast(F32R)
                )
                wqt_t.append(t)
            wkt_t = []
            for ss in range(KS):
                t = res.tile([128, INTER], F32R, tag=f"wkt{ss}", name=f"wkt{ss}")
                nc.sync.dma_start(
                    t[:], d_wkt[ss * 128:(ss + 1) * 128, :].bitcast(F32R)
                )
                wkt_t.append(t)
            wvt_t = []
            for ss in range(KS):
                t = res.tile([128, CS], F32R, tag=f"wvt{ss}", name=f"wvt{ss}")
                nc.sync.dma_start(
                    t[:], d_wvt[ss * 128:(ss + 1) * 128, :].bitcast(F32R)
                )
                wvt_t.append(t)
            bq_t = res.tile([INTER, 1], F32, tag="bq")
            nc.sync.dma_start(bq_t[:], d_bq[:])
            gam_t = res.tile([1, 1], F32, tag="gam")
            nc.sync.dma_start(gam_t[:], d_gam[:])
            ones_c = res.tile([128, 1], F32R, tag="ones_c")
            nc.sync.dma_start(ones_c[:], d_ones_c[:].bitcast(F32R))
            ones_r = res.tile([1, 128], F32R, tag="ones_r")
            nc.sync.dma_start(ones_r[:], d_ones_r[:].bitcast(F32R))
            # big activations: skip (feeds k and vT) before gate
            skip_t = []
            for ss in range(KS):
                t = res.tile([128, NJ], F32R, tag=f"skip{ss}", name=f"skip{ss}")
                nc.sync.dma_start(
                    t[:], d_skip[ss * 128:(ss + 1) * 128, :].bitcast(F32R)
                )
                skip_t.append(t)
            gate_t = []
            for kk in range(KG):
                t = res.tile([128, NI], F32R, tag=f"gate{kk}", name=f"gate{kk}")
                nc.sync.dma_start(
                    t[:], d_gate[kk * 128:(kk + 1) * 128, :].bitcast(F32R)
                )
                gate_t.append(t)
            # residual input is only needed by the epilogues
            skipr_t = []
            for ct in range(KS):
                t = res.tile([128, NI], F32, tag=f"skipr{ct}", name=f"skipr{ct}")
                nc.sync.dma_start(t[:], d_skipr[ct * 128:(ct + 1) * 128, :])
                skipr_t.append(t)

            q_parts = 128 if pair_st else INTER
            q_sb = res.tile([q_parts, NI], F32R, tag="q_sb")
            k_sb = res.tile([q_parts, NJ], F32R, tag="k_sb")
            vt_dt = BF16 if out_bf16 else F32R
            vt_sb = [
                res.tile([128, CS], vt_dt, tag=f"vt{jt}", name=f"vt{jt}")
                for jt in range(JT)
            ]
            if out_bf16:
                ones_cb = res.tile([128, 1], BF16, tag="ones_cb")
                nc.vector.tensor_copy(ones_cb[:], ones_c[:])

            # ---- projections ----
            import contextlib
            loop_ctx = tc.For_i(0, hw_loop, 1) if hw_loop else contextlib.nullcontext()
            proj_ctx = (tc.For_i(0, hw_loop_proj, 1)
                        if hw_loop_proj else contextlib.nullcontext())
            with loop_ctx:
              with proj_ctx:
               with tc.tile_pool(name="ps_proj", bufs=2, space="PSUM") as ps_proj:
                   # q[d,i] = sum_g WqT[g,d] gate[g,i]  (+bq later)
                   for n in range(NT):
                       pq = ps_proj.tile([INTER, 512], F32, tag="pq")
                       for kk in range(KG):
                           nc.tensor.matmul(
                               pq[:],
                               wqt_t[kk][:],
                               gate_t[kk][:, n * 512:(n + 1) * 512],
                               start=(kk == 0),
                               stop=(kk == KG - 1),
                           )
                       nc.vector.tensor_scalar(
                           q_sb[0:INTER, n * 512:(n + 1) * 512], pq[:],
                           bq_t[:, 0:1], None, op0=ALU.add,
                       )
                   # k[d,j] = sum_s WkT[s,d] skip[s,j]
                   for n in range(NJ // 512):
                       pk = ps_proj.tile([INTER, 512], F32, tag="pk")
                       for ss in range(KS):
                           nc.tensor.matmul(
                               pk[:],
                               wkt_t[ss][:],
                               skip_t[ss][:, n * 512:(n + 1) * 512],
                               start=(ss == 0),
                               stop=(ss == KS - 1),
                           )
                       nc.vector.tensor_copy(
                           k_sb[0:INTER, n * 512:(n + 1) * 512], pk[:]
                       )
                   if pair_st:
                       # duplicate q/k into partitions 64..127 for row-group
                       # paired logit matmuls
                       nc.sync.dma_start(q_sb[INTER:2 * INTER, :], q_sb[0:INTER, :])
                       nc.sync.dma_start(k_sb[INTER:2 * INTER, :], k_sb[0:INTER, :])
                   # vT[j,c] = sum_s skip[s,j] WvT[s,c]
                   for jt in range(JT):
                       pv = ps_proj.tile([128, CS], F32, tag="pv")
                       for ss in range(KS):
                           nc.tensor.matmul(
                               pv[:],
                               skip_t[ss][:, jt * 128:(jt + 1) * 128],
                               wvt_t[ss][:],
                               start=(ss == 0),
                               stop=(ss == KS - 1),
                           )
                       nc.vector.tensor_copy(vt_sb[jt][:], pv[:])

              # ---- attention, one 512-wide query stripe at a time ----
              with tc.tile_pool(name="ps_attn", bufs=1, space="PSUM") as ps:
                for _rep in range(repeat):
                  for n in range(NT if do_attn else 0):
                    inner_ctx = (tc.For_i(0, hw_loop_inner, 1)
                                 if hw_loop_inner else contextlib.nullcontext())
                    with inner_ctx:
                      qsl = q_sb[0:INTER, n * 512:(n + 1) * 512]
                      p_out = [
                          ps.tile([128, 512], F32, tag=f"out{ct}",
                                  name=f"p_out{ct}", bufs=2 if out_db else 1)
                          for ct in range(KS)
                      ]
                      p_sums = ps.tile([1, 512], F32, tag="sums")
                      if sums_on_dve:
                          acc = epi.tile([128, 512], F32R, tag="acc")
                      def emit_consumers(jg, P):
                          for u in range(exp_batch):
                              jt = jg * exp_batch + u
                              Pu = P[:, u * 512:(u + 1) * 512]
                              first = jt == 0
                              last = jt == JT - 1
                              for ct in range(KS):
                                  nc.tensor.matmul(
                                      p_out[ct][:],
                                      vt_sb[jt][:, ct * 128:(ct + 1) * 128],
                                      Pu,
                                      start=first,
                                      stop=last,
                                  )
                              if do_sums and not sums_on_dve:
                                  nc.tensor.matmul(
                                      p_sums[:],
                                      ones_cb[:] if out_bf16 else ones_c[:],
                                      Pu, start=first, stop=last,
                                  )
                              elif do_sums:
                                  if first:
                                      nc.vector.tensor_copy(acc[:], Pu)
                                  else:
                                      nc.vector.tensor_tensor(
                                          acc[:], acc[:], Pu, op=ALU.add
                                      )
                                  if last:
                                      nc.tensor.matmul(
                                          p_sums[:], ones_c[:], acc[:],
                                          start=True, stop=True,
                                      )

                      pending = []
                      for jg in range(JT // exp_batch):
                          # logits for exp_batch j-tiles into one tile
                          p_st = ps.tile(
                              [128, 512 * exp_batch], F32, tag="st",
                              bufs=st_bufs if exp_batch == 1 else {2: 2, 4: 1}[exp_batch],
                          )
                          for u in range(exp_batch):
                              jt = jg * exp_batch + u
                              if pair_st:
                                  half = jt % 2
                                  lo = half * INTER
                                  nc.tensor.matmul(
                                      p_st[:, u * 512:(u + 1) * 512],
                                      k_sb[lo:lo + INTER,
                                           jt * 128:(jt + 1) * 128],
                                      q_sb[lo:lo + INTER,
                                           n * 512:(n + 1) * 512],
                                      start=True,
                                      stop=True,
                                  )
                              else:
                                  nc.tensor.matmul(
                                      p_st[:, u * 512:(u + 1) * 512],
                                      k_sb[0:INTER, jt * 128:(jt + 1) * 128],
                                      qsl,
                                      start=True,
                                      stop=True,
                                  )
                          P = stream.tile([128, 512 * exp_batch],
                                          BF16 if out_bf16 else F32R, tag="P",
                                          bufs=p_bufs)
                          if no_exp:
                              nc.vector.tensor_copy(P[:], p_st[:])
                          else:
                              nc.scalar.activation(P[:], p_st[:], AF.Exp)
                          if sw_pipe:
                              pending.append((jg, P))
                              if len(pending) > sw_pipe:
                                  emit_consumers(*pending.pop(0))
                          else:
                              emit_consumers(jg, P)
                      for item in pending:
                          emit_consumers(*item)
                      # epilogue: out = (gamma/sums) * acc + (skip + gamma*bv)
                      rec = epi.tile([1, 512], F32, tag="rec")
                      if do_sums:
                          nc.vector.reciprocal(rec[:], p_sums[:])
                      else:
                          nc.vector.memset(rec[:], 1.0)
                      rg = epi.tile([1, 512], F32R, tag="rg")
                      nc.vector.tensor_scalar(
                          rg[:], rec[:], gam_t[0:1, 0:1], None, op0=ALU.mult
                      )
                      p_rb = ps.tile([128, 512], F32,
                                     tag="sums" if out_db else "rb")
                      nc.tensor.matmul(p_rb[:], ones_r[:], rg[:], start=True, stop=True)
                      rb_sb = epi.tile([128, 512], F32, tag="rb_sb")
                      nc.vector.tensor_copy(rb_sb[:], p_rb[:])
                      for ct in range(KS):
                          t0 = epi.tile([128, 512], F32, tag="t0")
                          nc.vector.tensor_tensor(
                              t0[:], p_out[ct][:], rb_sb[:], op=ALU.mult
                          )
                          out_t = epi.tile([128, 512], F32, tag="out_t")
                          nc.vector.tensor_tensor(
                              out_t[:],
                              t0[:],
                              skipr_t[ct][:, n * 512:(n + 1) * 512],
                              op=ALU.add,
                          )
                          nc.sync.dma_start(
                              d_out[ct * 128:(ct + 1) * 128, n * 512:(n + 1) * 512],
                              out_t[:],
                          )
    nc.compile()
    return nc


_PROGRAM_CACHE = None


def kernel(gate, skip, Wq, bq, Wk, bk, Wv, bv, gamma):
    global _PROGRAM_CACHE
    gate = np.ascontiguousarray(np.asarray(gate, dtype=np.float32)).reshape(B, CG, N)
    skip = np.ascontiguousarray(np.asarray(skip, dtype=np.float32)).reshape(B, CS, N)
    Wq = np.asarray(Wq, dtype=np.float32)
    bq = np.asarray(bq, dtype=np.float32)
    Wk = np.asarray(Wk, dtype=np.float32)
    Wv = np.asarray(Wv, dtype=np.float32)
    bv = np.asarray(bv, dtype=np.float32)
    gamma = np.asarray(gamma, dtype=np.float32)

    if _PROGRAM_CACHE is None:
        _PROGRAM_CACHE = _build_program()
    nc = _PROGRAM_CACHE

    wqt = np.ascontiguousarray(Wq.T)                  # [CG, INTER]
    wkt = np.ascontiguousarray(Wk.T)                  # [CS, INTER]
    wvt = np.ascontiguousarray(Wv.T)                  # [CS, CS]
    bq_c = np.ascontiguousarray(bq.reshape(INTER, 1))
    gam = gamma.reshape(1, 1)
    gbv = (gamma[0] * bv).reshape(CS, 1)
    ones_c = np.ones((128, 1), np.float32)
    ones_r = np.ones((1, 128), np.float32)

    in_maps = []
    for core in range(NCORES):
        b, h = divmod(core, 2)
        isl = slice(h * NI, (h + 1) * NI)
        in_maps.append(
            {
                "gate": np.ascontiguousarray(gate[b, :, isl]),
                "skip": skip[b],
                "skipr": np.ascontiguousarray(skip[b, :, isl]) + gbv,
                "wqt": wqt,
                "wkt": wkt,
                "wvt": wvt,
                "bq": bq_c,
                "gam": gam,
                "ones_c": ones_c,
                "ones_r": ones_r,
            }
        )

    res = run_bass_kernel_spmd(nc, in_maps, list(range(NCORES)))

    out = np.empty((B, CS, N), np.float32)
    for core in range(NCORES):
        b, h = divmod(core, 2)
        out[b, :, h * NI:(h + 1) * NI] = res.results[core]["out"]
    return out.reshape(B, CS, H, W)



# revision 20
# speedup vs baseline: 1.0185x; 1.0185x over previous
"""CrossAttentionGate kernel for Trainium2, 8 NeuronCores.

Problem: B=4 batches of single-head spatial cross-attention:
    q = Wq@gate + bq          [B,64,N]   (N = 64*64 = 4096)
    k = Wk@skip + bk          [B,64,N]
    v = Wv@skip + bv          [B,256,N]
    attn = softmax_j(q^T k)   [B,N,N]
    out = gamma * (v @ attn^T) + skip

Sharding: 8 cores = 4 batches x 2 query-halves. Each core computes its
batch's k/v in full (duplicated across the 2 cores of a batch - cheap)
and attends for its 2048 query positions.

Math simplifications used (exact, up to float rounding):
  - bk drops out: it shifts every logit in a row i by the same constant,
    softmax is invariant.
  - No row-max subtraction: logits are O(+-50) for this input
    distribution, exp() stays well inside fp32 range and the ACT engine's
    spline exp is accurate over that range (measured ~1e-5).
  - bv moves past the softmax: rows of attn sum to 1, so v's bias adds
    gamma*bv[c] to every output pixel - folded into the residual on host.

Layout trick: logits are computed TRANSPOSED (ST[j,i] = sum_d k[d,j]q[d,i])
so that P = exp(ST) feeds the output matmul as the moving operand with
j (the softmax axis) on partitions - no on-chip transpose anywhere.
Softmax denominators come from a ones-column matmul on the PE; the
1/sum (and gamma) scaling is applied to the 256x2048 output instead of
the 4096x2048 P matrix.

All matmuls run in float32r (reduced-precision fp32 PE mode, 4x faster
than fp32, ~1.6e-4 matmul rel err vs ~2.2e-3 for bf16).
"""

import numpy as np

import concourse.bass as bass
import concourse.tile as tile
from concourse import bacc, mybir
from concourse.bass_utils import run_bass_kernel_spmd

F32 = mybir.dt.float32
F32R = mybir.dt.float32r
AF = mybir.ActivationFunctionType
BF16 = mybir.dt.bfloat16
ALU = mybir.AluOpType

B, CG, CS, INTER, H, W = 4, 512, 256, 64, 64, 64
TPV = True  # transposed-PV kernel layout (out written as [NI, CS])
# best-known build config for the real kernel (and test.py's timing builds)
BEST = dict(p_bf16=True, sw_pipe=4, p_bufs=8, exp_batch=2)
N = H * W            # 4096 spatial positions
NCORES = 8
NI = N // 2          # 2048 query positions per core
NJ = N               # full key/value length per core

EXP_BATCH = 1        # ACT PSUM reads must stay within one PSUM bank


def _build_program(exp_batch=EXP_BATCH, do_sums=True, do_attn=True, repeat=1,
                   hw_loop=0, hw_loop_inner=0, hw_loop_proj=0, no_exp=False, st_bufs=4,
                   p_bufs=6, sums_on_dve=False, pair_st=True, sw_pipe=0, out_bf16=False, out_db=False,
                   proj_split=False, dup_in_copy=False, proj_repeat=1, tpv=None,
                   p_bf16=False):
    if tpv is None:
        tpv = TPV
    nc = bacc.Bacc(
        "TRN2", target_bir_lowering=False, debug=False, num_devices=NCORES
    )
    d_gate = nc.dram_tensor("gate", [CG, NI], F32, kind="ExternalInput").ap()
    d_skip = nc.dram_tensor("skip", [CS, NJ], F32, kind="ExternalInput").ap()
    if tpv:
        d_skipt = nc.dram_tensor("skipt", [NI, CS], F32, kind="ExternalInput").ap()
    else:
        d_skipr = nc.dram_tensor("skipr", [CS, NI], F32, kind="ExternalInput").ap()
    d_wqt = nc.dram_tensor("wqt", [CG, INTER], F32, kind="ExternalInput").ap()
    d_wkt = nc.dram_tensor("wkt", [CS, INTER], F32, kind="ExternalInput").ap()
    d_wvt = nc.dram_tensor("wvt", [CS, CS], F32, kind="ExternalInput").ap()
    d_bq = nc.dram_tensor("bq", [INTER, 1], F32, kind="ExternalInput").ap()
    d_gam = nc.dram_tensor("gam", [128, 1], F32, kind="ExternalInput").ap()
    d_ones_c = nc.dram_tensor("ones_c", [128, 1], F32, kind="ExternalInput").ap()
    d_ones_r = nc.dram_tensor("ones_r", [1, 128], F32, kind="ExternalInput").ap()
    if tpv:
        d_out = nc.dram_tensor("out", [NI, CS], F32, kind="ExternalOutput").ap()
    else:
        d_out = nc.dram_tensor("out", [CS, NI], F32, kind="ExternalOutput").ap()

    KG = CG // 128   # 4 gate channel tiles
    KS = CS // 128   # 2 skip channel tiles
    JT = NJ // 128   # 32 key tiles
    NT = NI // 512   # 4 query column tiles

    with tile.TileContext(nc) as tc:
        with (
            tc.tile_pool(name="res", bufs=1) as res,      # long-lived tensors
            tc.tile_pool(name="stream", bufs=4) as stream,  # P tiles etc.
            tc.tile_pool(name="epi", bufs=2) as epi,
        ):
            # ---- load everything (inputs cast to f32r via DMA bitcast) ----
            # weights first: they unblock the projection matmuls
            wqt_t = []
            for kk in range(KG):
                t = res.tile([128, INTER], F32R, tag=f"wqt{kk}", name=f"wqt{kk}")
                nc.sync.dma_start(
                    t[:], d_wqt[kk * 128:(kk + 1) * 128, :].bitcast(F32R)
                )
                wqt_t.append(t)
            wkt_t = []
            for ss in range(KS):
                t = res.tile([128, INTER], F32R, tag=f"wkt{ss}", name=f"wkt{ss}")
                nc.sync.dma_start(
                    t[:], d_wkt[ss * 128:(ss + 1) * 128, :].bitcast(F32R)
                )
                wkt_t.append(t)
            wvt_t = []
            for ss in range(KS):
                t = res.tile([128, CS], F32R, tag=f"wvt{ss}", name=f"wvt{ss}")
                nc.sync.dma_start(
                    t[:], d_wvt[ss * 128:(ss + 1) * 128, :].bitcast(F32R)
                )
                wvt_t.append(t)
            bq_t = res.tile([INTER, 1], F32, tag="bq")
            nc.sync.dma_start(bq_t[:], d_bq[:])
            gam_t = res.tile([128, 1], F32, tag="gam")
            nc.sync.dma_start(gam_t[:], d_gam[:])
            ones_c = res.tile([128, 1], F32R, tag="ones_c")
            nc.sync.dma_start(ones_c[:], d_ones_c[:].bitcast(F32R))
            ones_r = res.tile([1, 128], F32R, tag="ones_r")
            nc.sync.dma_start(ones_r[:], d_ones_r[:].bitcast(F32R))
            # big activations: skip (feeds k and vT) before gate
            skip_t = []
            for ss in range(KS):
                t = res.tile([128, NJ], F32R, tag=f"skip{ss}", name=f"skip{ss}")
                nc.sync.dma_start(
                    t[:], d_skip[ss * 128:(ss + 1) * 128, :].bitcast(F32R)
                )
                skip_t.append(t)
            gate_t = []
            for kk in range(KG):
                t = res.tile([128, NI], F32R, tag=f"gate{kk}", name=f"gate{kk}")
                nc.sync.dma_start(
                    t[:], d_gate[kk * 128:(kk + 1) * 128, :].bitcast(F32R)
                )
                gate_t.append(t)
            # residual input is only needed by the epilogues
            if tpv:
                skipt_t = []
                for it in range(NI // 128):
                    t = res.tile([128, CS], F32, tag=f"skipt{it}",
                                 name=f"skipt{it}")
                    nc.sync.dma_start(t[:], d_skipt[it * 128:(it + 1) * 128, :])
                    skipt_t.append(t)
            else:
                skipr_t = []
                for ct in range(KS):
                    t = res.tile([128, NI], F32, tag=f"skipr{ct}", name=f"skipr{ct}")
                    nc.sync.dma_start(t[:], d_skipr[ct * 128:(ct + 1) * 128, :])
                    skipr_t.append(t)

            q_parts = 128 if pair_st else INTER
            q_sb = res.tile([q_parts, NI], F32R, tag="q_sb")
            k_sb = res.tile([q_parts, NJ], F32R, tag="k_sb")
            assert not (tpv and out_bf16)
            vt_dt = BF16 if (out_bf16 or (tpv and p_bf16)) else F32R
            vt_w = CS + 2 if tpv else CS  # +2: ones col + even-width pad (fp32r needs even free size)
            vt_sb = [
                res.tile([128, vt_w], vt_dt, tag=f"vt{jt}", name=f"vt{jt}")
                for jt in range(JT)
            ]
            if tpv:
                # ones column: softmax denominator rides along as channel CS
                for jt in range(JT):
                    nc.vector.tensor_copy(vt_sb[jt][:, CS:CS + 1], ones_c[:])
                    nc.vector.tensor_copy(vt_sb[jt][:, CS + 1:CS + 2], ones_c[:])
            if out_bf16:
                ones_cb = res.tile([128, 1], BF16, tag="ones_cb")
                nc.vector.tensor_copy(ones_cb[:], ones_c[:])

            # ---- projections ----
            import contextlib
            loop_ctx = tc.For_i(0, hw_loop, 1) if hw_loop else contextlib.nullcontext()
            proj_ctx = (tc.For_i(0, hw_loop_proj, 1)
                        if hw_loop_proj else contextlib.nullcontext())
            with loop_ctx:
              with proj_ctx:
               for _prep in range(proj_repeat):
                with tc.tile_pool(name="ps_proj", bufs=2, space="PSUM") as ps_proj:
                   # q[d,i] = sum_g WqT[g,d] gate[g,i]  (+bq later)
                   for n in range(NT):
                       pq = ps_proj.tile([INTER, 512], F32, tag="pq")
                       for kk in range(KG):
                           nc.tensor.matmul(
                               pq[:],
                               wqt_t[kk][:],
                               gate_t[kk][:, n * 512:(n + 1) * 512],
                               start=(kk == 0),
                               stop=(kk == KG - 1),
                           )
                       qdst = q_sb[0:INTER, n * 512:(n + 1) * 512]
                       if proj_split:
                           nc.scalar.activation(
                               qdst, pq[:], AF.Identity, bias=bq_t[:, 0:1]
                           )
                       else:
                           nc.vector.tensor_scalar(
                               qdst, pq[:], bq_t[:, 0:1], None, op0=ALU.add,
                           )
                       if pair_st and dup_in_copy:
                           nc.vector.tensor_scalar(
                               q_sb[INTER:2 * INTER, n * 512:(n + 1) * 512],
                               pq[:], bq_t[:, 0:1], None, op0=ALU.add,
                           )
                   # k[d,j] = sum_s WkT[s,d] skip[s,j]
                   for n in range(NJ // 512):
                       pk = ps_proj.tile([INTER, 512], F32, tag="pk")
                       for ss in range(KS):
                           nc.tensor.matmul(
                               pk[:],
                               wkt_t[ss][:],
                               skip_t[ss][:, n * 512:(n + 1) * 512],
                               start=(ss == 0),
                               stop=(ss == KS - 1),
                           )
                       kdst = k_sb[0:INTER, n * 512:(n + 1) * 512]
                       if proj_split:
                           nc.scalar.activation(kdst, pk[:], AF.Copy)
                       else:
                           nc.vector.tensor_copy(kdst, pk[:])
                       if pair_st and dup_in_copy:
                           nc.vector.tensor_copy(
                               k_sb[INTER:2 * INTER, n * 512:(n + 1) * 512], pk[:]
                           )
                   if pair_st and not dup_in_copy:
                       # duplicate q/k into partitions 64..127 for row-group
                       # paired logit matmuls
                       nc.sync.dma_start(q_sb[INTER:2 * INTER, :], q_sb[0:INTER, :])
                       nc.sync.dma_start(k_sb[INTER:2 * INTER, :], k_sb[0:INTER, :])
                   # vT[j,c] = sum_s skip[s,j] WvT[s,c]
                   for jt in range(JT):
                       pv = ps_proj.tile([128, CS], F32, tag="pv")
                       for ss in range(KS):
                           nc.tensor.matmul(
                               pv[:],
                               skip_t[ss][:, jt * 128:(jt + 1) * 128],
                               wvt_t[ss][:],
                               start=(ss == 0),
                               stop=(ss == KS - 1),
                           )
                       if proj_split and jt % 2 == 0:
                           nc.scalar.activation(vt_sb[jt][:, 0:CS], pv[:], AF.Copy)
                       else:
                           nc.vector.tensor_copy(vt_sb[jt][:, 0:CS], pv[:])

              # ---- attention, one 512-wide query stripe at a time ----
              with tc.tile_pool(name="ps_attn", bufs=1, space="PSUM") as ps:
                for _rep in range(repeat):
                  for n in range(NT if do_attn else 0):
                    inner_ctx = (tc.For_i(0, hw_loop_inner, 1)
                                 if hw_loop_inner else contextlib.nullcontext())
                    with inner_ctx:
                      qsl = q_sb[0:INTER, n * 512:(n + 1) * 512]
                      if tpv:
                          p_ot = [
                              ps.tile([128, CS + 2], F32, tag=f"ot{ib}",
                                      name=f"p_ot{ib}")
                              for ib in range(4)
                          ]

                          p_dt = BF16 if p_bf16 else F32R

                          def emit_consumers(jg, P):
                              for u in range(exp_batch):
                                  jt = jg * exp_batch + u
                                  first = jt == 0
                                  last = jt == JT - 1
                                  for ib in range(4):
                                      nc.tensor.matmul(
                                          p_ot[ib][:],
                                          P[:, u * 512 + ib * 128:
                                            u * 512 + (ib + 1) * 128],
                                          vt_sb[jt][:],
                                          start=first,
                                          stop=last,
                                      )

                          pending = []
                          for jg in range(JT // exp_batch):
                              p_st = ps.tile(
                                  [128, 512 * exp_batch], F32, tag="st",
                                  bufs={1: st_bufs, 2: 2, 4: 1}[exp_batch],
                              )
                              for u in range(exp_batch):
                                  jt = jg * exp_batch + u
                                  if pair_st:
                                      lo = (jt % 2) * INTER
                                      nc.tensor.matmul(
                                          p_st[:, u * 512:(u + 1) * 512],
                                          k_sb[lo:lo + INTER,
                                               jt * 128:(jt + 1) * 128],
                                          q_sb[lo:lo + INTER,
                                               n * 512:(n + 1) * 512],
                                          start=True, stop=True,
                                      )
                                  else:
                                      nc.tensor.matmul(
                                          p_st[:, u * 512:(u + 1) * 512],
                                          k_sb[0:INTER,
                                               jt * 128:(jt + 1) * 128],
                                          qsl, start=True, stop=True,
                                      )
                              P = stream.tile([128, 512 * exp_batch], p_dt,
                                              tag="P", bufs=p_bufs)
                              nc.scalar.activation(P[:], p_st[:], AF.Exp)
                              if sw_pipe:
                                  pending.append((jg, P))
                                  if len(pending) > sw_pipe:
                                      emit_consumers(*pending.pop(0))
                              else:
                                  emit_consumers(jg, P)
                          for item in pending:
                              emit_consumers(*item)
                          # epilogue: out^T = (gamma/Z[i]) * acc^T + skip^T
                          for ib in range(4):
                              it = n * 4 + ib
                              rec = epi.tile([128, 1], F32, tag="rec")
                              nc.vector.reciprocal(
                                  rec[:], p_ot[ib][:, CS:CS + 1])
                              rg = epi.tile([128, 1], F32, tag="rg")
                              nc.vector.tensor_scalar(
                                  rg[:], rec[:], gam_t[:, 0:1], None,
                                  op0=ALU.mult,
                              )
                              t0 = epi.tile([128, CS], F32, tag="t0")
                              nc.vector.tensor_scalar(
                                  t0[:], p_ot[ib][:, 0:CS], rg[:, 0:1], None,
                                  op0=ALU.mult,
                              )
                              out_t = epi.tile([128, CS], F32, tag="out_t")
                              nc.vector.tensor_tensor(
                                  out_t[:], t0[:], skipt_t[it][:], op=ALU.add,
                              )
                              nc.sync.dma_start(
                                  d_out[it * 128:(it + 1) * 128, :], out_t[:],
                              )
                          continue
                      p_out = [
                          ps.tile([128, 512], F32, tag=f"out{ct}",
                                  name=f"p_out{ct}", bufs=2 if out_db else 1)
                          for ct in range(KS)
                      ]
                      p_sums = ps.tile([1, 512], F32, tag="sums")
                      if sums_on_dve:
                          acc = epi.tile([128, 512], F32R, tag="acc")
                      def emit_consumers(jg, P):
                          for u in range(exp_batch):
                              jt = jg * exp_batch + u
                              Pu = P[:, u * 512:(u + 1) * 512]
                              first = jt == 0
                              last = jt == JT - 1
                              for ct in range(KS):
                                  nc.tensor.matmul(
                                      p_out[ct][:],
                                      vt_sb[jt][:, ct * 128:(ct + 1) * 128],
                                      Pu,
                                      start=first,
                                      stop=last,
                                  )
                              if do_sums and not sums_on_dve:
                                  nc.tensor.matmul(
                                      p_sums[:],
                                      ones_cb[:] if out_bf16 else ones_c[:],
                                      Pu, start=first, stop=last,
                                  )
                              elif do_sums:
                                  if first:
                                      nc.vector.tensor_copy(acc[:], Pu)
                                  else:
                                      nc.vector.tensor_tensor(
                                          acc[:], acc[:], Pu, op=ALU.add
                                      )
                                  if last:
                                      nc.tensor.matmul(
                                          p_sums[:], ones_c[:], acc[:],
                                          start=True, stop=True,
                                      )

                      pending = []
                      for jg in range(JT // exp_batch):
                          # logits for exp_batch j-tiles into one tile
                          p_st = ps.tile(
                              [128, 512 * exp_batch], F32, tag="st",
                              bufs=st_bufs if exp_batch == 1 else {2: 2, 4: 1}[exp_batch],
                          )
                          for u in range(exp_batch):
                              jt = jg * exp_batch + u
                              if pair_st:
                                  half = jt % 2
                                  lo = half * INTER
                                  nc.tensor.matmul(
                                      p_st[:, u * 512:(u + 1) * 512],
                                      k_sb[lo:lo + INTER,
                                           jt * 128:(jt + 1) * 128],
                                      q_sb[lo:lo + INTER,
                                           n * 512:(n + 1) * 512],
                                      start=True,
                                      stop=True,
                                  )
                              else:
                                  nc.tensor.matmul(
                                      p_st[:, u * 512:(u + 1) * 512],
                                      k_sb[0:INTER, jt * 128:(jt + 1) * 128],
                                      qsl,
                                      start=True,
                                      stop=True,
                                  )
                          P = stream.tile([128, 512 * exp_batch],
                                          BF16 if out_bf16 else F32R, tag="P",
                                          bufs=p_bufs)
                          if no_exp:
                              nc.vector.tensor_copy(P[:], p_st[:])
                          else:
                              nc.scalar.activation(P[:], p_st[:], AF.Exp)
                          if sw_pipe:
                              pending.append((jg, P))
                              if len(pending) > sw_pipe:
                                  emit_consumers(*pending.pop(0))
                          else:
                              emit_consumers(jg, P)
                      for item in pending:
                          emit_consumers(*item)
                      # epilogue: out = (gamma/sums) * acc + (skip + gamma*bv)
                      rec = epi.tile([1, 512], F32, tag="rec")
                      if do_sums:
                          nc.vector.reciprocal(rec[:], p_sums[:])
                      else:
                          nc.vector.memset(rec[:], 1.0)
                      rg = epi.tile([1, 512], F32R, tag="rg")
                      nc.vector.tensor_scalar(
                          rg[:], rec[:], gam_t[0:1, 0:1], None, op0=ALU.mult
                      )
                      p_rb = ps.tile([128, 512], F32,
                                     tag="sums" if out_db else "rb")
                      nc.tensor.matmul(p_rb[:], ones_r[:], rg[:], start=True, stop=True)
                      rb_sb = epi.tile([128, 512], F32, tag="rb_sb")
                      nc.vector.tensor_copy(rb_sb[:], p_rb[:])
                      for ct in range(KS):
                          t0 = epi.tile([128, 512], F32, tag="t0")
                          nc.vector.tensor_tensor(
                              t0[:], p_out[ct][:], rb_sb[:], op=ALU.mult
                          )
                          out_t = epi.tile([128, 512], F32, tag="out_t")
                          nc.vector.tensor_tensor(
                              out_t[:],
                              t0[:],
                              skipr_t[ct][:, n * 512:(n + 1) * 512],
                              op=ALU.add,
                          )
                          nc.sync.dma_start(
                              d_out[ct * 128:(ct + 1) * 128, n * 512:(n + 1) * 512],
                              out_t[:],
                          )
    nc.compile()
    return nc


_PROGRAM_CACHE = None


def make_in_maps(gate, skip, Wq, bq, Wk, bk, Wv, bv, gamma):
    gate = np.ascontiguousarray(np.asarray(gate, dtype=np.float32)).reshape(B, CG, N)
    skip = np.ascontiguousarray(np.asarray(skip, dtype=np.float32)).reshape(B, CS, N)
    Wq = np.asarray(Wq, dtype=np.float32)
    bq = np.asarray(bq, dtype=np.float32)
    Wk = np.asarray(Wk, dtype=np.float32)
    Wv = np.asarray(Wv, dtype=np.float32)
    bv = np.asarray(bv, dtype=np.float32)
    gamma = np.asarray(gamma, dtype=np.float32)

    wqt = np.ascontiguousarray(Wq.T)                  # [CG, INTER]
    wkt = np.ascontiguousarray(Wk.T)                  # [CS, INTER]
    wvt = np.ascontiguousarray(Wv.T)                  # [CS, CS]
    bq_c = np.ascontiguousarray(bq.reshape(INTER, 1))
    gam = np.full((128, 1), gamma[0], np.float32)
    gbv = (gamma[0] * bv).reshape(CS, 1)
    ones_c = np.ones((128, 1), np.float32)
    ones_r = np.ones((1, 128), np.float32)

    in_maps = []
    for core in range(NCORES):
        b, h = divmod(core, 2)
        isl = slice(h * NI, (h + 1) * NI)
        skipr = np.ascontiguousarray(skip[b, :, isl]) + gbv
        in_maps.append(
            {
                "gate": np.ascontiguousarray(gate[b, :, isl]),
                "skip": skip[b],
                "skipr": skipr,
                "skipt": np.ascontiguousarray(skipr.T),
                "wqt": wqt,
                "wkt": wkt,
                "wvt": wvt,
                "bq": bq_c,
                "gam": gam,
                "ones_c": ones_c,
                "ones_r": ones_r,
            }
        )
    return in_maps


def kernel(gate, skip, Wq, bq, Wk, bk, Wv, bv, gamma):
    global _PROGRAM_CACHE
    if _PROGRAM_CACHE is None:
        _PROGRAM_CACHE = _build_program(**BEST)
    nc = _PROGRAM_CACHE

    in_maps = make_in_maps(gate, skip, Wq, bq, Wk, bk, Wv, bv, gamma)
    res = run_bass_kernel_spmd(nc, in_maps, list(range(NCORES)))

    out = np.empty((B, CS, N), np.float32)
    for core in range(NCORES):
        b, h = divmod(core, 2)
        o = res.results[core]["out"]
        out[b, :, h * NI:(h + 1) * NI] = o.T if TPV else o
    return out.reshape(B, CS, H, W)



# revision 24
# speedup vs baseline: 1.4879x; 1.4608x over previous
"""CrossAttentionGate kernel for Trainium2, 8 NeuronCores.

Problem: B=4 batches of single-head spatial cross-attention:
    q = Wq@gate + bq          [B,64,N]   (N = 64*64 = 4096)
    k = Wk@skip + bk          [B,64,N]
    v = Wv@skip + bv          [B,256,N]
    attn = softmax_j(q^T k)   [B,N,N]
    out = gamma * (v @ attn^T) + skip

Sharding: 8 cores = 4 batches x 2 query-halves. Each core computes its
batch's k/v in full (duplicated across the 2 cores of a batch - cheap)
and attends for its 2048 query positions.

Math simplifications used (exact, up to float rounding):
  - bk drops out: it shifts every logit in a row i by the same constant,
    softmax is invariant.
  - No row-max subtraction: logits are O(+-50) for this input
    distribution, exp() stays well inside fp32 range and the ACT engine's
    spline exp is accurate over that range (measured ~1e-5).
  - bv moves past the softmax: rows of attn sum to 1, so v's bias adds
    gamma*bv[c] to every output pixel - folded into the residual on host.

Layout trick: logits are computed TRANSPOSED (ST[j,i] = sum_d k[d,j]q[d,i])
so that P = exp(ST) feeds the output matmul as the moving operand with
j (the softmax axis) on partitions - no on-chip transpose anywhere.
Softmax denominators come from a ones-column matmul on the PE; the
1/sum (and gamma) scaling is applied to the 256x2048 output instead of
the 4096x2048 P matrix.

All matmuls run in float32r (reduced-precision fp32 PE mode, 4x faster
than fp32, ~1.6e-4 matmul rel err vs ~2.2e-3 for bf16).
"""

import numpy as np

import concourse.bass as bass
import concourse.tile as tile
from concourse import bacc, mybir
from concourse.bass_utils import run_bass_kernel_spmd

F32 = mybir.dt.float32
F32R = mybir.dt.float32r
AF = mybir.ActivationFunctionType
BF16 = mybir.dt.bfloat16
ALU = mybir.AluOpType

B, CG, CS, INTER, H, W = 4, 512, 256, 64, 64, 64
TPV = True  # transposed-PV kernel layout (out written as [NI, CS])
# best-known build config for the real kernel (and test.py's timing builds)
BEST = dict(p_bf16=True, sw_pipe=4, p_bufs=8, exp_batch=2)
N = H * W            # 4096 spatial positions
NCORES = 8
NI = N // 2          # 2048 query positions per core
NJ = N               # full key/value length per core

EXP_BATCH = 1        # ACT PSUM reads must stay within one PSUM bank


def _build_program(exp_batch=EXP_BATCH, do_sums=True, do_attn=True, repeat=1,
                   hw_loop=0, hw_loop_inner=0, hw_loop_proj=0, no_exp=False, st_bufs=4,
                   p_bufs=6, sums_on_dve=False, pair_st=True, sw_pipe=0, out_bf16=False, out_db=False,
                   proj_split=False, dup_in_copy=False, proj_repeat=1, tpv=None,
                   p_bf16=False, fake_p=False, no_st=False, proj_pool=False):
    if tpv is None:
        tpv = TPV
    nc = bacc.Bacc(
        "TRN2", target_bir_lowering=False, debug=False, num_devices=NCORES
    )
    d_gate = nc.dram_tensor("gate", [CG, NI], F32, kind="ExternalInput").ap()
    d_skip = nc.dram_tensor("skip", [CS, NJ], F32, kind="ExternalInput").ap()
    if tpv:
        d_skipt = nc.dram_tensor("skipt", [NI, CS], F32, kind="ExternalInput").ap()
    else:
        d_skipr = nc.dram_tensor("skipr", [CS, NI], F32, kind="ExternalInput").ap()
    d_wqt = nc.dram_tensor("wqt", [CG, INTER], F32, kind="ExternalInput").ap()
    d_wkt = nc.dram_tensor("wkt", [CS, INTER], F32, kind="ExternalInput").ap()
    d_wvt = nc.dram_tensor("wvt", [CS, CS], F32, kind="ExternalInput").ap()
    d_bq = nc.dram_tensor("bq", [INTER, 1], F32, kind="ExternalInput").ap()
    d_gam = nc.dram_tensor("gam", [128, 1], F32, kind="ExternalInput").ap()
    d_ones_c = nc.dram_tensor("ones_c", [128, 1], F32, kind="ExternalInput").ap()
    d_ones_r = nc.dram_tensor("ones_r", [1, 128], F32, kind="ExternalInput").ap()
    if tpv:
        d_out = nc.dram_tensor("out", [NI, CS], F32, kind="ExternalOutput").ap()
    else:
        d_out = nc.dram_tensor("out", [CS, NI], F32, kind="ExternalOutput").ap()

    KG = CG // 128   # 4 gate channel tiles
    KS = CS // 128   # 2 skip channel tiles
    JT = NJ // 128   # 32 key tiles
    NT = NI // 512   # 4 query column tiles

    with tile.TileContext(nc) as tc:
        with (
            tc.tile_pool(name="res", bufs=1) as res,      # long-lived tensors
            tc.tile_pool(name="stream", bufs=4) as stream,  # P tiles etc.
            tc.tile_pool(name="epi", bufs=2) as epi,
        ):
            # ---- load everything (inputs cast to f32r via DMA bitcast) ----
            # weights first: they unblock the projection matmuls
            wqt_t = []
            for kk in range(KG):
                t = res.tile([128, INTER], F32R, tag=f"wqt{kk}", name=f"wqt{kk}")
                nc.sync.dma_start(
                    t[:], d_wqt[kk * 128:(kk + 1) * 128, :].bitcast(F32R)
                )
                wqt_t.append(t)
            wkt_t = []
            for ss in range(KS):
                t = res.tile([128, INTER], F32R, tag=f"wkt{ss}", name=f"wkt{ss}")
                nc.sync.dma_start(
                    t[:], d_wkt[ss * 128:(ss + 1) * 128, :].bitcast(F32R)
                )
                wkt_t.append(t)
            wvt_t = []
            for ss in range(KS):
                t = res.tile([128, CS], F32R, tag=f"wvt{ss}", name=f"wvt{ss}")
                nc.sync.dma_start(
                    t[:], d_wvt[ss * 128:(ss + 1) * 128, :].bitcast(F32R)
                )
                wvt_t.append(t)
            bq_t = res.tile([INTER, 1], F32, tag="bq")
            nc.sync.dma_start(bq_t[:], d_bq[:])
            gam_t = res.tile([128, 1], F32, tag="gam")
            nc.sync.dma_start(gam_t[:], d_gam[:])
            ones_c = res.tile([128, 1], F32R, tag="ones_c")
            nc.sync.dma_start(ones_c[:], d_ones_c[:].bitcast(F32R))
            ones_r = res.tile([1, 128], F32R, tag="ones_r")
            nc.sync.dma_start(ones_r[:], d_ones_r[:].bitcast(F32R))
            # big activations: skip (feeds k and vT) before gate
            skip_t = []
            for ss in range(KS):
                t = res.tile([128, NJ], F32R, tag=f"skip{ss}", name=f"skip{ss}")
                nc.sync.dma_start(
                    t[:], d_skip[ss * 128:(ss + 1) * 128, :].bitcast(F32R)
                )
                skip_t.append(t)
            gate_t = []
            for kk in range(KG):
                t = res.tile([128, NI], F32R, tag=f"gate{kk}", name=f"gate{kk}")
                nc.sync.dma_start(
                    t[:], d_gate[kk * 128:(kk + 1) * 128, :].bitcast(F32R)
                )
                gate_t.append(t)
            # residual input is only needed by the epilogues
            if tpv:
                skipt_t = []
                for it in range(NI // 128):
                    t = res.tile([128, CS], F32, tag=f"skipt{it}",
                                 name=f"skipt{it}")
                    nc.sync.dma_start(t[:], d_skipt[it * 128:(it + 1) * 128, :])
                    skipt_t.append(t)
            else:
                skipr_t = []
                for ct in range(KS):
                    t = res.tile([128, NI], F32, tag=f"skipr{ct}", name=f"skipr{ct}")
                    nc.sync.dma_start(t[:], d_skipr[ct * 128:(ct + 1) * 128, :])
                    skipr_t.append(t)

            q_parts = 128 if pair_st else INTER
            q_sb = res.tile([q_parts, NI], F32R, tag="q_sb")
            k_sb = res.tile([q_parts, NJ], F32R, tag="k_sb")
            assert not (tpv and out_bf16)
            vt_dt = BF16 if (out_bf16 or (tpv and p_bf16)) else F32R
            vt_w = CS + 2 if tpv else CS  # +2: ones col + even-width pad (fp32r needs even free size)
            vt_sb = [
                res.tile([128, vt_w], vt_dt, tag=f"vt{jt}", name=f"vt{jt}")
                for jt in range(JT)
            ]
            if tpv:
                # ones column: softmax denominator rides along as channel CS
                for jt in range(JT):
                    nc.vector.tensor_copy(vt_sb[jt][:, CS:CS + 1], ones_c[:])
                    nc.vector.tensor_copy(vt_sb[jt][:, CS + 1:CS + 2], ones_c[:])
            if out_bf16:
                ones_cb = res.tile([128, 1], BF16, tag="ones_cb")
                nc.vector.tensor_copy(ones_cb[:], ones_c[:])

            # ---- projections ----
            import contextlib
            loop_ctx = tc.For_i(0, hw_loop, 1) if hw_loop else contextlib.nullcontext()
            proj_ctx = (tc.For_i(0, hw_loop_proj, 1)
                        if hw_loop_proj else contextlib.nullcontext())
            with loop_ctx:
              with proj_ctx:
               for _prep in range(proj_repeat):
                with tc.tile_pool(name="ps_proj", bufs=2, space="PSUM") as ps_proj:
                   # q[d,i] = sum_g WqT[g,d] gate[g,i]  (+bq later)
                   for n in range(NT):
                       pq = ps_proj.tile([INTER, 512], F32, tag="pq")
                       for kk in range(KG):
                           nc.tensor.matmul(
                               pq[:],
                               wqt_t[kk][:],
                               gate_t[kk][:, n * 512:(n + 1) * 512],
                               start=(kk == 0),
                               stop=(kk == KG - 1),
                           )
                       qdst = q_sb[0:INTER, n * 512:(n + 1) * 512]
                       if proj_split:
                           nc.scalar.activation(
                               qdst, pq[:], AF.Identity, bias=bq_t[:, 0:1]
                           )
                       else:
                           nc.vector.tensor_scalar(
                               qdst, pq[:], bq_t[:, 0:1], None, op0=ALU.add,
                           )
                       if pair_st and dup_in_copy:
                           nc.vector.tensor_scalar(
                               q_sb[INTER:2 * INTER, n * 512:(n + 1) * 512],
                               pq[:], bq_t[:, 0:1], None, op0=ALU.add,
                           )
                   # k[d,j] = sum_s WkT[s,d] skip[s,j]
                   for n in range(NJ // 512):
                       pk = ps_proj.tile([INTER, 512], F32, tag="pk")
                       for ss in range(KS):
                           nc.tensor.matmul(
                               pk[:],
                               wkt_t[ss][:],
                               skip_t[ss][:, n * 512:(n + 1) * 512],
                               start=(ss == 0),
                               stop=(ss == KS - 1),
                           )
                       kdst = k_sb[0:INTER, n * 512:(n + 1) * 512]
                       if proj_split:
                           nc.scalar.activation(kdst, pk[:], AF.Copy)
                       else:
                           nc.vector.tensor_copy(kdst, pk[:])
                       if pair_st and dup_in_copy:
                           nc.vector.tensor_copy(
                               k_sb[INTER:2 * INTER, n * 512:(n + 1) * 512], pk[:]
                           )
                   if pair_st and not dup_in_copy:
                       # duplicate q/k into partitions 64..127 for row-group
                       # paired logit matmuls
                       nc.sync.dma_start(q_sb[INTER:2 * INTER, :], q_sb[0:INTER, :])
                       nc.sync.dma_start(k_sb[INTER:2 * INTER, :], k_sb[0:INTER, :])
                   # vT[j,c] = sum_s skip[s,j] WvT[s,c]
                   for jt in range(JT):
                       pv = ps_proj.tile([128, CS], F32, tag="pv")
                       for ss in range(KS):
                           nc.tensor.matmul(
                               pv[:],
                               skip_t[ss][:, jt * 128:(jt + 1) * 128],
                               wvt_t[ss][:],
                               start=(ss == 0),
                               stop=(ss == KS - 1),
                           )
                       if proj_split and jt % 2 == 0:
                           nc.scalar.activation(vt_sb[jt][:, 0:CS], pv[:], AF.Copy)
                       elif proj_pool and jt % 2 == 0:
                           nc.gpsimd.tensor_copy(vt_sb[jt][:, 0:CS], pv[:])
                       else:
                           nc.vector.tensor_copy(vt_sb[jt][:, 0:CS], pv[:])

              # ---- attention, one 512-wide query stripe at a time ----
              with tc.tile_pool(name="ps_attn", bufs=1, space="PSUM") as ps:
                for _rep in range(repeat):
                  for n in range(NT if do_attn else 0):
                    inner_ctx = (tc.For_i(0, hw_loop_inner, 1)
                                 if hw_loop_inner else contextlib.nullcontext())
                    with inner_ctx:
                      qsl = q_sb[0:INTER, n * 512:(n + 1) * 512]
                      if tpv:
                          p_ot = [
                              ps.tile([128, CS + 2], F32, tag=f"ot{ib}",
                                      name=f"p_ot{ib}")
                              for ib in range(4)
                          ]

                          p_dt = BF16 if p_bf16 else F32R

                          def emit_consumers(jg, P):
                              for u in range(exp_batch):
                                  jt = jg * exp_batch + u
                                  first = jt == 0
                                  last = jt == JT - 1
                                  for ib in range(4):
                                      nc.tensor.matmul(
                                          p_ot[ib][:],
                                          P[:, u * 512 + ib * 128:
                                            u * 512 + (ib + 1) * 128],
                                          vt_sb[jt][:],
                                          start=first,
                                          stop=last,
                                      )

                          fake_tiles = []
                          if fake_p:
                              p_fk = ps.tile(
                                  [128, 512 * exp_batch], F32, tag="st",
                                  bufs={1: st_bufs, 2: 2, 4: 1}[exp_batch],
                              )
                              nc.tensor.matmul(
                                  p_fk[:, 0:512],
                                  k_sb[0:INTER, 0:128],
                                  qsl, start=True, stop=True,
                              )
                              for b in range(p_bufs):
                                  Pf = stream.tile(
                                      [128, 512 * exp_batch], p_dt,
                                      tag="Pf", bufs=p_bufs, name=f"Pf{b}")
                                  nc.scalar.activation(Pf[:], p_fk[:], AF.Exp)
                                  fake_tiles.append(Pf)
                          pending = []
                          for jg in range(JT // exp_batch):
                              if not no_st:
                                p_st = ps.tile(
                                    [128, 512 * exp_batch], F32, tag="st",
                                    bufs={1: st_bufs, 2: 2, 4: 1}[exp_batch],
                                )
                                for u in range(exp_batch):
                                  jt = jg * exp_batch + u
                                  if pair_st:
                                      lo = (jt % 2) * INTER
                                      nc.tensor.matmul(
                                          p_st[:, u * 512:(u + 1) * 512],
                                          k_sb[lo:lo + INTER,
                                               jt * 128:(jt + 1) * 128],
                                          q_sb[lo:lo + INTER,
                                               n * 512:(n + 1) * 512],
                                          start=True, stop=True,
                                      )
                                  else:
                                      nc.tensor.matmul(
                                          p_st[:, u * 512:(u + 1) * 512],
                                          k_sb[0:INTER,
                                               jt * 128:(jt + 1) * 128],
                                          qsl, start=True, stop=True,
                                      )
                              if fake_p:
                                  emit_consumers(jg, fake_tiles[jg % p_bufs])
                                  continue
                              P = stream.tile([128, 512 * exp_batch], p_dt,
                                              tag="P", bufs=p_bufs)
                              nc.scalar.activation(P[:], p_st[:], AF.Exp)
                              if sw_pipe:
                                  pending.append((jg, P))
                                  if len(pending) > sw_pipe:
                                      emit_consumers(*pending.pop(0))
                              else:
                                  emit_consumers(jg, P)
                          for item in pending:
                              emit_consumers(*item)
                          # epilogue: out^T = (gamma/Z[i]) * acc^T + skip^T
                          for ib in range(4):
                              it = n * 4 + ib
                              rec = epi.tile([128, 1], F32, tag="rec")
                              nc.vector.reciprocal(
                                  rec[:], p_ot[ib][:, CS:CS + 1])
                              rg = epi.tile([128, 1], F32, tag="rg")
                              nc.vector.tensor_scalar(
                                  rg[:], rec[:], gam_t[:, 0:1], None,
                                  op0=ALU.mult,
                              )
                              t0 = epi.tile([128, CS], F32, tag="t0")
                              nc.vector.tensor_scalar(
                                  t0[:], p_ot[ib][:, 0:CS], rg[:, 0:1], None,
                                  op0=ALU.mult,
                              )
                              out_t = epi.tile([128, CS], F32, tag="out_t")
                              nc.vector.tensor_tensor(
                                  out_t[:], t0[:], skipt_t[it][:], op=ALU.add,
                              )
                              nc.sync.dma_start(
                                  d_out[it * 128:(it + 1) * 128, :], out_t[:],
                              )
                          continue
                      p_out = [
                          ps.tile([128, 512], F32, tag=f"out{ct}",
                                  name=f"p_out{ct}", bufs=2 if out_db else 1)
                          for ct in range(KS)
                      ]
                      p_sums = ps.tile([1, 512], F32, tag="sums")
                      if sums_on_dve:
                          acc = epi.tile([128, 512], F32R, tag="acc")
                      def emit_consumers(jg, P):
                          for u in range(exp_batch):
                              jt = jg * exp_batch + u
                              Pu = P[:, u * 512:(u + 1) * 512]
                              first = jt == 0
                              last = jt == JT - 1
                              for ct in range(KS):
                                  nc.tensor.matmul(
                                      p_out[ct][:],
                                      vt_sb[jt][:, ct * 128:(ct + 1) * 128],
                                      Pu,
                                      start=first,
                                      stop=last,
                                  )
                              if do_sums and not sums_on_dve:
                                  nc.tensor.matmul(
                                      p_sums[:],
                                      ones_cb[:] if out_bf16 else ones_c[:],
                                      Pu, start=first, stop=last,
                                  )
                              elif do_sums:
                                  if first:
                                      nc.vector.tensor_copy(acc[:], Pu)
                                  else:
                                      nc.vector.tensor_tensor(
                                          acc[:], acc[:], Pu, op=ALU.add
                                      )
                                  if last:
                                      nc.tensor.matmul(
                                          p_sums[:], ones_c[:], acc[:],
                                          start=True, stop=True,
                                      )

                      pending = []
                      for jg in range(JT // exp_batch):
                          # logits for exp_batch j-tiles into one tile
                          p_st = ps.tile(
                              [128, 512 * exp_batch], F32, tag="st",
                              bufs=st_bufs if exp_batch == 1 else {2: 2, 4: 1}[exp_batch],
                          )
                          for u in range(exp_batch):
                              jt = jg * exp_batch + u
                              if pair_st:
                                  half = jt % 2
                                  lo = half * INTER
                                  nc.tensor.matmul(
                                      p_st[:, u * 512:(u + 1) * 512],
                                      k_sb[lo:lo + INTER,
                                           jt * 128:(jt + 1) * 128],
                                      q_sb[lo:lo + INTER,
                                           n * 512:(n + 1) * 512],
                                      start=True,
                                      stop=True,
                                  )
                              else:
                                  nc.tensor.matmul(
                                      p_st[:, u * 512:(u + 1) * 512],
                                      k_sb[0:INTER, jt * 128:(jt + 1) * 128],
                                      qsl,
                                      start=True,
                                      stop=True,
                                  )
                          P = stream.tile([128, 512 * exp_batch],
                                          BF16 if out_bf16 else F32R, tag="P",
                                          bufs=p_bufs)
                          if no_exp:
                              nc.vector.tensor_copy(P[:], p_st[:])
                          else:
                              nc.scalar.activation(P[:], p_st[:], AF.Exp)
                          if sw_pipe:
                              pending.append((jg, P))
                              if len(pending) > sw_pipe:
                                  emit_consumers(*pending.pop(0))
                          else:
                              emit_consumers(jg, P)
                      for item in pending:
                          emit_consumers(*item)
                      # epilogue: out = (gamma/sums) * acc + (skip + gamma*bv)
                      rec = epi.tile([1, 512], F32, tag="rec")
                      if do_sums:
                          nc.vector.reciprocal(rec[:], p_sums[:])
                      else:
                          nc.vector.memset(rec[:], 1.0)
                      rg = epi.tile([1, 512], F32R, tag="rg")
                      nc.vector.tensor_scalar(
                          rg[:], rec[:], gam_t[0:1, 0:1], None, op0=ALU.mult
                      )
                      p_rb = ps.tile([128, 512], F32,
                                     tag="sums" if out_db else "rb")
                      nc.tensor.matmul(p_rb[:], ones_r[:], rg[:], start=True, stop=True)
                      rb_sb = epi.tile([128, 512], F32, tag="rb_sb")
                      nc.vector.tensor_copy(rb_sb[:], p_rb[:])
                      for ct in range(KS):
                          t0 = epi.tile([128, 512], F32, tag="t0")
                          nc.vector.tensor_tensor(
                              t0[:], p_out[ct][:], rb_sb[:], op=ALU.mult
                          )
                          out_t = epi.tile([128, 512], F32, tag="out_t")
                          nc.vector.tensor_tensor(
                              out_t[:],
                              t0[:],
                              skipr_t[ct][:, n * 512:(n + 1) * 512],
                              op=ALU.add,
                          )
                          nc.sync.dma_start(
                              d_out[ct * 128:(ct + 1) * 128, n * 512:(n + 1) * 512],
                              out_t[:],
                          )
    nc.compile()
    return nc


_PROGRAM_CACHE = None


def make_in_maps(gate, skip, Wq, bq, Wk, bk, Wv, bv, gamma):
    gate = np.ascontiguousarray(np.asarray(gate, dtype=np.float32)).reshape(B, CG, N)
    skip = np.ascontiguousarray(np.asarray(skip, dtype=np.float32)).reshape(B, CS, N)
    Wq = np.asarray(Wq, dtype=np.float32)
    bq = np.asarray(bq, dtype=np.float32)
    Wk = np.asarray(Wk, dtype=np.float32)
    Wv = np.asarray(Wv, dtype=np.float32)
    bv = np.asarray(bv, dtype=np.float32)
    gamma = np.asarray(gamma, dtype=np.float32)

    wqt = np.ascontiguousarray(Wq.T)                  # [CG, INTER]
    wkt = np.ascontiguousarray(Wk.T)                  # [CS, INTER]
    wvt = np.ascontiguousarray(Wv.T)                  # [CS, CS]
    bq_c = np.ascontiguousarray(bq.reshape(INTER, 1))
    gam = np.full((128, 1), gamma[0], np.float32)
    gbv = (gamma[0] * bv).reshape(CS, 1)
    ones_c = np.ones((128, 1), np.float32)
    ones_r = np.ones((1, 128), np.float32)

    in_maps = []
    for core in range(NCORES):
        b, h = divmod(core, 2)
        isl = slice(h * NI, (h + 1) * NI)
        skipr = np.ascontiguousarray(skip[b, :, isl]) + gbv
        in_maps.append(
            {
                "gate": np.ascontiguousarray(gate[b, :, isl]),
                "skip": skip[b],
                "skipr": skipr,
                "skipt": np.ascontiguousarray(skipr.T),
                "wqt": wqt,
                "wkt": wkt,
                "wvt": wvt,
                "bq": bq_c,
                "gam": gam,
                "ones_c": ones_c,
                "ones_r": ones_r,
            }
        )
    return in_maps


def kernel(gate, skip, Wq, bq, Wk, bk, Wv, bv, gamma):
    global _PROGRAM_CACHE
    if _PROGRAM_CACHE is None:
        _PROGRAM_CACHE = _build_program(**BEST)
    nc = _PROGRAM_CACHE

    in_maps = make_in_maps(gate, skip, Wq, bq, Wk, bk, Wv, bv, gamma)
    res = run_bass_kernel_spmd(nc, in_maps, list(range(NCORES)))

    out = np.empty((B, CS, N), np.float32)
    for core in range(NCORES):
        b, h = divmod(core, 2)
        o = res.results[core]["out"]
        out[b, :, h * NI:(h + 1) * NI] = o.T if TPV else o
    return out.reshape(B, CS, H, W)

